# revision 17
# baseline (speedup 1.0000x reference)
"""Trainium2 Bass kernel for an 8-expert top-2 MoE layer (B=8,S=2048,D=256,F=1024).

Strategy: data-parallel over the 16384 tokens across 8 NeuronCores (2048
tokens/core). Per core:
  - router in fp32 (matmul -> softmax-top2 via Max8/MatchReplace -> renorm)
  - shared expert + private experts as bf16 matmuls with fp32 PSUM accum
  - MODE="dense": all 8 experts computed for every token, masked combine.
  - MODE="sparse": tokens are compacted per expert on-device (triangular-matmul
    prefix sums -> slot addresses -> indirect-DMA scatter of x rows), each
    expert processes only its own tokens, combine gathers the two expert rows
    per token and applies the renormalized router weights.
All activations flow transposed ([feature, token]) so no on-chip transposes
are needed except the DMA-transpose load of gathered x in sparse mode.
"""

import sys

sys.path.insert(0, "/opt/trn_rl_repo")

import numpy as np
import ml_dtypes

import concourse.bass as bass
import concourse.mybir as mybir
import concourse.tile as tile
from concourse.bass import IndirectOffsetOnAxis
from concourse.bass_utils import run_bass_kernel_spmd


# ---------------------------------------------------------------------------
# Workaround: this walrus build supports only ONE semaphore wait per
# instruction ("Too many sync wait commands"). After Tile scheduling, split
# any instruction with N>1 waits into N-1 preceding same-engine NoOps that
# carry one wait each (stream order within the block preserves semantics).


def _split_multi_waits(nc):
    for fn in nc.m.functions:
        for bb in fn.blocks:
            insts = list(bb.instructions)
            out = []
            changed = False
            for inst in insts:
                si = inst.sync_info
                if si is not None and len(si.on_wait) > 1:
                    waits = list(si.on_wait)
                    for w in waits[:-1]:
                        nop = mybir.InstNoOp(
                            name=nc.get_next_instruction_name(),
                            engine=inst.engine,
                            bass_nofuse=True,
                            ins=[],
                            outs=[],
                        )
                        nop.sync_info = mybir.SyncInfo(on_wait=[w], on_update=[])
                        out.append(nop)
                    inst.sync_info = mybir.SyncInfo(
                        on_wait=[waits[-1]], on_update=list(si.on_update)
                    )
                    changed = True
                out.append(inst)
            if changed:
                bb.instructions = out
BF16 = mybir.dt.bfloat16
F32 = mybir.dt.float32
I32 = mybir.dt.int32
AF = mybir.ActivationFunctionType
ALU = mybir.AluOpType
AX = mybir.AxisListType

# problem dims
B, S, D, F, E, K = 8, 2048, 256, 1024, 8, 2
NCORES = 8
T = B * S // NCORES          # tokens per core = 2048
NT = T // 128                # token tiles per core = 16
KD = D // 128                # k-tiles over D = 2
NF = F // 128                # f-tiles over F = 8
CAP = 768                    # per-expert token capacity (mean is 512)
NS = CAP // 128              # slot tiles per expert = 6

MODE = "sparse"              # "dense" or "sparse"

bf16 = ml_dtypes.bfloat16


def _to_bf(a):
    return np.ascontiguousarray(a.astype(bf16))


def _to_f32(a):
    return np.ascontiguousarray(a.astype(np.float32))


def make_core_inputs(x_shard, w):
    """x_shard: [T, D] fp32. w: dict of full weight arrays. Returns in_map."""
    xT = np.ascontiguousarray(x_shard.T)                     # [D, T]
    xt32 = xT.reshape(KD, 128, T).transpose(1, 0, 2).reshape(128, KD * T)
    m = {
        "xt32": _to_f32(xt32),
        "xtb": _to_bf(xt32),
        "wr_t": _to_f32(w["Wr"].reshape(KD, 128, E).transpose(1, 0, 2).reshape(128, KD * E)),
        "w1s_t": _to_bf(w["W1s"].reshape(KD, 128, F).transpose(1, 0, 2).reshape(128, KD * F)),
        "w2s_t": _to_bf(w["W2s"].reshape(NF, 128, D).transpose(1, 0, 2).reshape(128, NF * D)),
        "w1p_t": _to_bf(w["W1p"].reshape(E, KD, 128, F).transpose(2, 0, 1, 3).reshape(128, E * KD * F)),
        "w2p_t": _to_bf(w["W2p"].reshape(E, NF, 128, D).transpose(2, 0, 1, 3).reshape(128, E * NF * D)),
        "b1s_t": _to_f32(w["b1s"].reshape(NF, 128).T),
        "b1p_t": _to_f32(w["b1p"].reshape(E, NF, 128).transpose(2, 0, 1).reshape(128, E * NF)),
        "b2s_r": _to_bf(w["b2s"].reshape(1, D)),
        "b2p_r": _to_bf(w["b2p"].reshape(1, E * D)),
        "ones_b": np.ones((1, 128), dtype=bf16),
    }
    if MODE == "sparse":
        m["xb"] = _to_bf(x_shard)                            # [T, D] token-major
        m["ones_f"] = np.ones((1, 128), dtype=np.float32)
        m["onec_f"] = np.ones((128, 1), dtype=np.float32)
        ks, ms = np.meshgrid(np.arange(128), np.arange(128), indexing="ij")
        m["tri_s"] = _to_f32(ks < ms)                        # lhsT[k,m]=1 iff k<m
        m["ecp1"] = _to_f32(np.broadcast_to(np.arange(E) * CAP + 1.0, (128, E)))
    return m


def build_nc(body_reps=1):
    nc = bass.Bass()
    # inputs
    xt32 = nc.declare_dram_parameter("xt32", [128, KD * T], F32, isOutput=False)
    xtb = nc.declare_dram_parameter("xtb", [128, KD * T], BF16, isOutput=False)
    wr_t = nc.declare_dram_parameter("wr_t", [128, KD * E], F32, isOutput=False)
    w1s_t = nc.declare_dram_parameter("w1s_t", [128, KD * F], BF16, isOutput=False)
    w2s_t = nc.declare_dram_parameter("w2s_t", [128, NF * D], BF16, isOutput=False)
    w1p_t = nc.declare_dram_parameter("w1p_t", [128, E * KD * F], BF16, isOutput=False)
    w2p_t = nc.declare_dram_parameter("w2p_t", [128, E * NF * D], BF16, isOutput=False)
    b1s_t = nc.declare_dram_parameter("b1s_t", [128, NF], F32, isOutput=False)
    b1p_t = nc.declare_dram_parameter("b1p_t", [128, E * NF], F32, isOutput=False)
    b2s_r = nc.declare_dram_parameter("b2s_r", [1, D], BF16, isOutput=False)
    b2p_r = nc.declare_dram_parameter("b2p_r", [1, E * D], BF16, isOutput=False)
    ones_b = nc.declare_dram_parameter("ones_b", [1, 128], BF16, isOutput=False)
    if MODE == "sparse":
        xb = nc.declare_dram_parameter("xb", [T, D], BF16, isOutput=False)
        ones_f = nc.declare_dram_parameter("ones_f", [1, 128], F32, isOutput=False)
        onec_f = nc.declare_dram_parameter("onec_f", [128, 1], F32, isOutput=False)
        tri_s = nc.declare_dram_parameter("tri_s", [128, 128], F32, isOutput=False)
        ecp1 = nc.declare_dram_parameter("ecp1", [128, E], F32, isOutput=False)
    y = nc.declare_dram_parameter("y", [T, D], F32, isOutput=True)

    with tile.TileContext(nc) as tc:
        with (
            tc.tile_pool(name="const", bufs=1) as cpool,
            tc.tile_pool(name="hts", bufs=1) as hts_pool,
            tc.tile_pool(name="hte", bufs=(2 if MODE == "sparse" else 1)) as hte_pool,
            tc.tile_pool(name="acc", bufs=1) as acc_pool,
            tc.tile_pool(name="rsm", bufs=2) as rsm,
            tc.tile_pool(name="psr", bufs=3, space="PSUM") as psr,
            tc.tile_pool(name="ps1", bufs=1, space="PSUM") as ps1,
            tc.tile_pool(name="psh", bufs=2, space="PSUM") as psh,
            tc.tile_pool(name="psc", bufs=2, space="PSUM") as psc,
            tc.tile_pool(name="outp", bufs=3) as outp,
        ):
            # ---- load constants / weights / activations into SBUF
            def load(pool, src, shape, dtype):
                t = pool.tile(shape, dtype, tag=f"c_{src.name}")
                nc.sync.dma_start(t[:], src[:])
                return t

            xt32_s = load(cpool, xt32, [128, KD * T], F32)
            xtb_s = load(cpool, xtb, [128, KD * T], BF16)
            wr_s = load(cpool, wr_t, [128, KD * E], F32)
            w1s_s = load(cpool, w1s_t, [128, KD * F], BF16)
            w2s_s = load(cpool, w2s_t, [128, NF * D], BF16)
            w1p_s = load(cpool, w1p_t, [128, E * KD * F], BF16)
            w2p_s = load(cpool, w2p_t, [128, E * NF * D], BF16)
            b1s_s = load(cpool, b1s_t, [128, NF], F32)
            b1p_s = load(cpool, b1p_t, [128, E * NF], F32)
            b2s_s = load(cpool, b2s_r, [1, D], BF16)
            b2p_s = load(cpool, b2p_r, [1, E * D], BF16)
            ones_s = load(cpool, ones_b, [1, 128], BF16)
            if MODE == "sparse":
                onesf_s = load(cpool, ones_f, [1, 128], F32)
                onecf_s = load(cpool, onec_f, [128, 1], F32)
                tri_s_s = load(cpool, tri_s, [128, 128], F32)
                ecp1_s = load(cpool, ecp1, [128, E], F32)
                # DRAM scratch
                with tc.tile_pool(name="dscr", bufs=1, space="DRAM") as dpool:
                    xg_d = dpool.tile([E * CAP, D], BF16)
                    eo_d = dpool.tile([E * CAP, D], BF16)
                    env = locals()
                    for _rep in range(body_reps):
                        run_sparse(nc, tc, env)
            else:
                env = locals()
                for _rep in range(body_reps):
                    run_dense(nc, tc, env)
    _split_multi_waits(nc)
    return nc


def router_tile(nc, t, env, rsm, want_dense_wn):
    """Router for token tile t. Returns dict with per-tile router tiles."""
    psr = env["psr"]
    xt32_s, wr_s = env["xt32_s"], env["wr_s"]
    pr = psr.tile([128, E], F32, tag="pr")
    for kd in range(KD):
        nc.tensor.matmul(
            pr[:],
            xt32_s[:, kd * T + t * 128 : kd * T + (t + 1) * 128],
            wr_s[:, kd * E : (kd + 1) * E],
            start=(kd == 0),
            stop=(kd == KD - 1),
        )
    negmax = rsm.tile([128, 1], F32, tag="negmax")
    nc.vector.tensor_reduce(negmax[:], pr[:], axis=AX.X, op=ALU.max)
    nc.vector.tensor_scalar_mul(negmax[:], negmax[:], -1.0)
    u = rsm.tile([128, E], F32, tag="u")
    nc.scalar.activation(u[:], pr[:], AF.Exp, bias=negmax[:, 0:1])
    s8 = rsm.tile([128, 8], F32, tag="s8")
    nc.vector.max(out=s8[:], in_=u[:])
    nc.vector.memset(s8[:, K:8], 0.0)
    zap = rsm.tile([128, E], F32, tag="zap")
    nc.vector.match_replace(out=zap[:], in_to_replace=s8[:], in_values=u[:], imm_value=0.0)
    wm = rsm.tile([128, E], F32, tag="wm")
    nc.vector.tensor_sub(wm[:], u[:], zap[:])
    out = {"wm": wm}
    if want_dense_wn:
        ssum = rsm.tile([128, 1], F32, tag="ssum")
        nc.vector.tensor_reduce(ssum[:], wm[:], axis=AX.X, op=ALU.add)
        rs = rsm.tile([128, 1], F32, tag="rs")
        nc.vector.reciprocal(rs[:], ssum[:])
        wn = rsm.tile([128, E], F32, tag="wn")
        nc.vector.tensor_scalar(wn[:], wm[:], rs[:, 0:1], None, op0=ALU.mult)
        out["wn"] = wn
    return out


def shared_l1(nc, env):
    """Shared expert first layer -> hTs [128f, NF x T] bf16 (persistent)."""
    psh, hts_pool = env["psh"], env["hts_pool"]
    xtb_s, w1s_s, b1s_s = env["xtb_s"], env["w1s_s"], env["b1s_s"]
    hts = hts_pool.tile([128, NF * T], BF16)
    for c in range(T // 512):
        for j in range(NF):
            ph = psh.tile([128, 512], F32, tag="ph")
            for kd in range(KD):
                nc.tensor.matmul(
                    ph[:],
                    w1s_s[:, kd * F + j * 128 : kd * F + (j + 1) * 128],
                    xtb_s[:, kd * T + c * 512 : kd * T + (c + 1) * 512],
                    start=(kd == 0),
                    stop=(kd == KD - 1),
                )
            nc.scalar.activation(
                hts[:, j * T + c * 512 : j * T + c * 512 + 512],
                ph[:],
                AF.Gelu,
                bias=b1s_s[:, j : j + 1],
            )
    return hts


def shared_l2_tile(nc, env, hts, t):
    """Shared expert second layer for token tile t -> psum [128tok, D]."""
    psc = env["psc"]
    w2s_s, b2s_s, ones_s = env["w2s_s"], env["b2s_s"], env["ones_s"]
    pc = psc.tile([128, D], F32, tag="pc")
    for j in range(NF):
        nc.tensor.matmul(
            pc[:],
            hts[:, j * T + t * 128 : j * T + (t + 1) * 128],
            w2s_s[:, j * D : (j + 1) * D],
            start=(j == 0),
            stop=False,
        )
    nc.tensor.matmul(pc[:], ones_s[:, :], b2s_s[:, :], start=False, stop=True)
    return pc


def run_dense(nc, tc, env):
    psh, psc = env["psh"], env["psc"]
    rsm, outp = env["rsm"], env["outp"]
    acc_pool, hte_pool = env["acc_pool"], env["hte_pool"]
    xtb_s = env["xtb_s"]
    w1p_s, w2p_s, b1p_s, b2p_s = env["w1p_s"], env["w2p_s"], env["b1p_s"], env["b2p_s"]
    ones_s = env["ones_s"]
    y = env["y"]

    # router for all tiles; keep dense renormalized weights
    wn_all = acc_pool.tile([128, NT * E], F32, tag="wn_all")
    for t in range(NT):
        r = router_tile(nc, t, env, rsm, want_dense_wn=True)
        nc.vector.tensor_copy(wn_all[:, t * E : (t + 1) * E], r["wn"][:])

    hts = shared_l1(nc, env)
    acc = acc_pool.tile([128, NT * D], F32, tag="acc")
    for t in range(NT):
        pc = shared_l2_tile(nc, env, hts, t)
        nc.vector.tensor_copy(acc[:, t * D : (t + 1) * D], pc[:])

    for e in range(E):
        hte = hte_pool.tile([128, NF * T], BF16, tag="hte")
        for c in range(T // 512):
            for j in range(NF):
                ph = psh.tile([128, 512], F32, tag="ph")
                for kd in range(KD):
                    nc.tensor.matmul(
                        ph[:],
                        w1p_s[:, (e * KD + kd) * F + j * 128 : (e * KD + kd) * F + (j + 1) * 128],
                        xtb_s[:, kd * T + c * 512 : kd * T + (c + 1) * 512],
                        start=(kd == 0),
                        stop=(kd == KD - 1),
                    )
                nc.scalar.activation(
                    hte[:, j * T + c * 512 : j * T + c * 512 + 512],
                    ph[:],
                    AF.Gelu,
                    bias=b1p_s[:, e * NF + j : e * NF + j + 1],
                )
        for t in range(NT):
            pe = psc.tile([128, D], F32, tag="pc")
            for j in range(NF):
                nc.tensor.matmul(
                    pe[:],
                    hte[:, j * T + t * 128 : j * T + (t + 1) * 128],
                    w2p_s[:, (e * NF + j) * D : (e * NF + j + 1) * D],
                    start=(j == 0),
                    stop=False,
                )
            nc.tensor.matmul(pe[:], ones_s[:, :], b2p_s[0:1, e * D : (e + 1) * D], start=False, stop=True)
            # acc += wn[:, e] * pe
            nc.vector.scalar_tensor_tensor(
                out=acc[:, t * D : (t + 1) * D],
                in0=pe[:],
                scalar=wn_all[:, t * E + e : t * E + e + 1],
                in1=acc[:, t * D : (t + 1) * D],
                op0=ALU.mult,
                op1=ALU.add,
            )
    for t in range(NT):
        nc.sync.dma_start(y[t * 128 : (t + 1) * 128, :], acc[:, t * D : (t + 1) * D])


def run_sparse(nc, tc, env):
    psh, psc, psr = env["psh"], env["psc"], env["psr"]
    rsm, outp = env["rsm"], env["outp"]
    acc_pool, hte_pool = env["acc_pool"], env["hte_pool"]
    xtb_s = env["xtb_s"]
    w1p_s, w2p_s, b1p_s, b2p_s = env["w1p_s"], env["w2p_s"], env["b1p_s"], env["b2p_s"]
    ones_s, onesf_s, onecf_s = env["ones_s"], env["onesf_s"], env["onecf_s"]
    tri_s_s, ecp1_s = env["tri_s_s"], env["ecp1_s"]
    xb, xg_d, eo_d = env["xb"], env["xg_d"], env["eo_d"]
    y = env["y"]

    # persistent per-tile router outputs
    wn01 = acc_pool.tile([128, NT * 2], F32, tag="wn01")      # renorm weights slot0/1
    ai = acc_pool.tile([128, NT * 2], I32, tag="ai")          # slot addresses slot0/1
    m_all = acc_pool.tile([128, NT * E], F32, tag="m_all")    # top2 selection masks
    wm_all = acc_pool.tile([128, NT * E], F32, tag="wm_all")  # masked unnorm weights
    pin_all = acc_pool.tile([128, NT * E], F32, tag="pin_all")  # intra pos + e*CAP + 1
    crow = acc_pool.tile([1, NT * E], F32, tag="crow")        # per-tile expert counts
    bases = acc_pool.tile([1, NT * E], F32, tag="bases")      # exclusive tile bases

    # pass A: router + intra-tile positions
    for t in range(NT):
        r = router_tile(nc, t, env, rsm, want_dense_wn=False)
        wm = r["wm"]
        me = m_all[:, t * E : (t + 1) * E]
        nc.vector.tensor_scalar(me, wm[:], 0.0, None, op0=ALU.is_gt)
        nc.vector.tensor_copy(wm_all[:, t * E : (t + 1) * E], wm[:])
        pp = psr.tile([128, E], F32, tag="pr")
        nc.tensor.matmul(pp[:], tri_s_s[:, :], me, start=True, stop=True)
        # pin = intra_pos + e*CAP + 1
        nc.vector.tensor_add(pin_all[:, t * E : (t + 1) * E], pp[:], ecp1_s[:])

    # per-tile expert counts in one matmul, then DVE-only prefix chain
    pcnt = env["ps1"].tile([1, NT * E], F32, tag="pcnt")
    nc.tensor.matmul(pcnt[:], onecf_s[:, :], m_all[:, :], start=True, stop=True)
    nc.vector.tensor_copy(crow[:], pcnt[:])
    nc.vector.memset(bases[:, 0:E], 0.0)
    for t in range(1, NT):
        nc.vector.tensor_add(
            bases[:, t * E : (t + 1) * E],
            bases[:, (t - 1) * E : t * E],
            crow[:, (t - 1) * E : t * E],
        )

    # pass B: global addresses, top-2 slot extraction, x-row scatter
    bc_reg = nc.gpsimd.to_reg(E * CAP - 1)
    for t in range(NT):
        bb = psr.tile([128, E], F32, tag="pr")
        nc.tensor.matmul(
            bb[:], onesf_s[:, :], bases[0:1, t * E : (t + 1) * E], start=True, stop=True
        )
        addr1 = rsm.tile([128, E], F32, tag="addr1")
        nc.vector.tensor_add(addr1[:], pin_all[:, t * E : (t + 1) * E], bb[:])
        key1 = rsm.tile([128, E], F32, tag="key1")
        nc.vector.tensor_mul(key1[:], m_all[:, t * E : (t + 1) * E], addr1[:])
        key2 = rsm.tile([128, E], F32, tag="key2")
        nc.vector.scalar_tensor_tensor(
            out=key2[:],
            in0=wm_all[:, t * E : (t + 1) * E],
            scalar=0.5,
            in1=key1[:],
            op0=ALU.mult,
            op1=ALU.add,
        )
        s1 = rsm.tile([128, 8], F32, tag="s1")
        s2 = rsm.tile([128, 8], F32, tag="s2")
        nc.vector.max(out=s1[:], in_=key1[:])
        nc.vector.max(out=s2[:], in_=key2[:])
        dk = rsm.tile([128, 2], F32, tag="dk")
        nc.vector.tensor_sub(dk[:], s2[:, 0:2], s1[:, 0:2])
        ssum = rsm.tile([128, 1], F32, tag="ssum")
        nc.vector.tensor_add(ssum[:], dk[:, 0:1], dk[:, 1:2])
        rs = rsm.tile([128, 1], F32, tag="rs")
        nc.vector.reciprocal(rs[:], ssum[:])
        nc.vector.tensor_scalar(
            wn01[:, t * 2 : t * 2 + 2], dk[:], rs[:, 0:1], None, op0=ALU.mult
        )
        a01 = rsm.tile([128, 2], F32, tag="a01")
        nc.vector.tensor_scalar(a01[:], s1[:, 0:2], 1.0, None, op0=ALU.subtract)
        nc.vector.tensor_copy(ai[:, t * 2 : t * 2 + 2], a01[:])
        # scatter this tile's x rows into both expert slots
        xrow = outp.tile([128, D], BF16, tag="xrow")
        nc.sync.dma_start(xrow[:], xb[t * 128 : (t + 1) * 128, :])
        for k in range(2):
            nc.gpsimd.indirect_dma_start(
                out=xg_d[:],
                out_offset=IndirectOffsetOnAxis(ap=ai[:, t * 2 + k : t * 2 + k + 1], axis=0),
                in_=xrow[:],
                in_offset=None,
                bounds_check=bc_reg,
                oob_is_err=True,
            )

    # shared expert L1 (overlaps with router/scatter on other engines)
    hts = shared_l1(nc, env)

    # private experts on gathered tokens
    for e in range(E):
        xgt = hte_pool.tile([128, KD * CAP], BF16, tag="xgt")
        for kd in range(KD):
            nc.sync.dma_start(
                xgt[:, kd * CAP : (kd + 1) * CAP],
                xg_d[e * CAP : (e + 1) * CAP, kd * 128 : (kd + 1) * 128],
                transpose=True,
            )
        hte = hte_pool.tile([128, NF * CAP], BF16, tag="hte")
        for c0 in range(0, CAP, 512):
            cn = min(512, CAP - c0)
            for j in range(NF):
                ph = psh.tile([128, 512], F32, tag="ph")
                for kd in range(KD):
                    nc.tensor.matmul(
                        ph[:, 0:cn],
                        w1p_s[:, (e * KD + kd) * F + j * 128 : (e * KD + kd) * F + (j + 1) * 128],
                        xgt[:, kd * CAP + c0 : kd * CAP + c0 + cn],
                        start=(kd == 0),
                        stop=(kd == KD - 1),
                    )
                nc.scalar.activation(
                    hte[:, j * CAP + c0 : j * CAP + c0 + cn],
                    ph[:, 0:cn],
                    AF.Gelu,
                    bias=b1p_s[:, e * NF + j : e * NF + j + 1],
                )
        for s in range(NS):
            pe = psc.tile([128, D], F32, tag="pc")
            for j in range(NF):
                nc.tensor.matmul(
                    pe[:],
                    hte[:, j * CAP + s * 128 : j * CAP + (s + 1) * 128],
                    w2p_s[:, (e * NF + j) * D : (e * NF + j + 1) * D],
                    start=(j == 0),
                    stop=False,
                )
            nc.tensor.matmul(pe[:], ones_s[:, :], b2p_s[0:1, e * D : (e + 1) * D], start=False, stop=True)
            eo = outp.tile([128, D], BF16, tag="eo")
            nc.vector.tensor_copy(eo[:], pe[:])
            nc.sync.dma_start(eo_d[e * CAP + s * 128 : e * CAP + (s + 1) * 128, :], eo[:])

    # combine: shared L2 + two gathered expert rows per token
    for t in range(NT):
        pc = shared_l2_tile(nc, env, hts, t)
        g0 = outp.tile([128, D], BF16, tag="g0")
        g1 = outp.tile([128, D], BF16, tag="g1")
        nc.gpsimd.indirect_dma_start(
            out=g0[:],
            out_offset=None,
            in_=eo_d[:],
            in_offset=IndirectOffsetOnAxis(ap=ai[:, t * 2 : t * 2 + 1], axis=0),
        )
        nc.gpsimd.indirect_dma_start(
            out=g1[:],
            out_offset=None,
            in_=eo_d[:],
            in_offset=IndirectOffsetOnAxis(ap=ai[:, t * 2 + 1 : t * 2 + 2], axis=0),
        )
        c1 = outp.tile([128, D], F32, tag="c1")
        nc.vector.scalar_tensor_tensor(
            out=c1[:], in0=g0[:], scalar=wn01[:, t * 2 : t * 2 + 1],
            in1=pc[:], op0=ALU.mult, op1=ALU.add,
        )
        ot = outp.tile([128, D], F32, tag="ot")
        nc.vector.scalar_tensor_tensor(
            out=ot[:], in0=g1[:], scalar=wn01[:, t * 2 + 1 : t * 2 + 2],
            in1=c1[:], op0=ALU.mult, op1=ALU.add,
        )
        nc.sync.dma_start(y[t * 128 : (t + 1) * 128, :], ot[:])


_NC_CACHE = {}


def _get_nc(body_reps=1):
    if body_reps not in _NC_CACHE:
        _NC_CACHE[body_reps] = build_nc(body_reps)
    return _NC_CACHE[body_reps]


def _make_in_maps(inputs):
    x = np.asarray(inputs["x"], dtype=np.float32).reshape(B * S, D)
    w = {k: np.asarray(v, dtype=np.float32) for k, v in inputs.items() if k != "x"}
    return [make_core_inputs(x[i * T : (i + 1) * T], w) for i in range(NCORES)]


def run(inputs, trace=False):
    nc = _get_nc()
    in_maps = _make_in_maps(inputs)
    res = run_bass_kernel_spmd(nc, in_maps, list(range(NCORES)), trace=trace)
    out = np.concatenate([res.results[i]["y"] for i in range(NCORES)], axis=0)
    return out.reshape(B, S, D), res



def bench(inputs, iters=8, reps=3, nc=None, in_maps=None, body_reps=1):
    """Marginal per-execution device time: `iters` chained executions
    (outputs donated forward), minus per-call dispatch measured separately."""
    import time as _time

    import jax
    import numpy as _np
    from jax.experimental.shard_map import shard_map
    from jax.sharding import Mesh, PartitionSpec

    from concourse import bass2jax

    if nc is None:
        nc = _get_nc(body_reps)
    if in_maps is None:
        in_maps = _make_in_maps(inputs)
    n_cores = NCORES

    in_names, out_names, out_avals, zero_outs = [], [], [], []
    for alloc in nc.m.functions[0].allocations:
        if not isinstance(alloc, mybir.MemoryLocationSet):
            continue
        name = alloc.memorylocations[0].name
        if alloc.kind == "ExternalInput":
            if nc.partition_id_tensor is None or name != nc.partition_id_tensor.name:
                in_names.append(name)
        elif alloc.kind == "ExternalOutput":
            shape = tuple(alloc.tensor_shape)
            dtype = mybir.dt.np(alloc.dtype)
            out_names.append(name)
            out_avals.append(jax.core.ShapedArray(shape, dtype))
            zero_outs.append(_np.zeros(shape, dtype))
    n_params = len(in_names)
    all_names = in_names + out_names
    if nc.partition_id_tensor is not None:
        all_names = all_names + [nc.partition_id_tensor.name]

    def _body(*args):
        ops = list(args)
        ins, outs = ops[:n_params], ops[n_params:]
        pid = (
            [bass2jax.partition_id_tensor()]
            if nc.partition_id_tensor is not None
            else []
        )
        outs = list(
            bass2jax._bass_exec_p.bind(
                *ins,
                *outs,
                *pid,
                out_avals=tuple(out_avals),
                in_names=tuple(all_names),
                out_names=tuple(out_names),
                lowering_input_output_aliases=(),
                sim_require_finite=True,
                sim_require_nnan=True,
                nc=nc,
            )
        )
        return tuple(outs)

    devices = jax.devices()[:n_cores]
    mesh = Mesh(_np.asarray(devices), ("core",))
    nin = n_params + len(zero_outs)
    fn = jax.jit(
        shard_map(
            _body,
            mesh=mesh,
            in_specs=(PartitionSpec("core"),) * nin,
            out_specs=(PartitionSpec("core"),) * len(out_names),
            check_rep=False,
        ),
        donate_argnums=tuple(range(n_params, nin)),
        keep_unused=True,
    )
    concat_in = [
        _np.concatenate([_np.asarray(in_maps[c][k]) for c in range(n_cores)], axis=0)
        for k in in_names
    ]
    dev_in = [jax.device_put(a) for a in concat_in]
    times = []
    for _ in range(reps + 1):
        outs = [
            _np.zeros((n_cores * z.shape[0], *z.shape[1:]), z.dtype) for z in zero_outs
        ]
        jax.block_until_ready(dev_in)
        t0 = _time.perf_counter()
        for _i in range(iters):
            outs = list(fn(*dev_in, *outs))
        jax.block_until_ready(outs)
        times.append(_time.perf_counter() - t0)
    return min(times[1:]), [_np.asarray(o) for o in outs]


def make_tiny_nc():
    nc = bass.Bass()
    a = nc.declare_dram_parameter("a", [128, 8], F32, isOutput=False)
    z = nc.declare_dram_parameter("z", [128, 8], F32, isOutput=True)
    with tile.TileContext(nc) as tc:
        with tc.tile_pool(name="t", bufs=1) as tp:
            ta = tp.tile([128, 8], F32, tag="ta")
            nc.sync.dma_start(ta[:], a[:])
            nc.sync.dma_start(z[:], ta[:])
    _split_multi_waits(nc)
    return nc


def bench_floor(iters=8, reps=3):
    import numpy as _np

    nc = make_tiny_nc()
    maps = [{"a": _np.zeros((128, 8), _np.float32)} for _ in range(NCORES)]
    t, _ = bench(None, iters=iters, reps=reps, nc=nc, in_maps=maps)
    return t


def kernel(**inputs):
    out, _ = run(inputs, trace=False)
    return out


# revision 26
# speedup vs baseline: 6.5921x; 6.5921x over previous
"""Trainium2 Bass kernel for an 8-expert top-2 MoE layer (B=8,S=2048,D=256,F=1024).

Strategy: data-parallel over the 16384 tokens across 8 NeuronCores (2048
tokens/core). Per core:
  - router in fp32 (matmul -> softmax-top2 via Max8/MatchReplace -> renorm)
  - shared expert + private experts as bf16 matmuls with fp32 PSUM accum
  - MODE="dense": all 8 experts computed for every token, masked combine.
  - MODE="sparse": tokens are compacted per expert on-device (triangular-matmul
    prefix sums -> slot addresses -> indirect-DMA scatter of x rows), each
    expert processes only its own tokens, combine gathers the two expert rows
    per token and applies the renormalized router weights.
All activations flow transposed ([feature, token]) so no on-chip transposes
are needed except the DMA-transpose load of gathered x in sparse mode.
"""

import sys

sys.path.insert(0, "/opt/trn_rl_repo")

import numpy as np
import ml_dtypes

import concourse.bass as bass
import concourse.mybir as mybir
import concourse.tile as tile
from concourse.bass import IndirectOffsetOnAxis
from concourse.bass_utils import run_bass_kernel_spmd


# ---------------------------------------------------------------------------
# Workaround: this walrus build supports only ONE semaphore wait per
# instruction ("Too many sync wait commands"). After Tile scheduling, split
# any instruction with N>1 waits into N-1 preceding same-engine NoOps that
# carry one wait each (stream order within the block preserves semantics).


def _split_multi_waits(nc):
    for fn in nc.m.functions:
        for bb in fn.blocks:
            insts = list(bb.instructions)
            out = []
            changed = False
            for inst in insts:
                si = inst.sync_info
                if si is not None and len(si.on_wait) > 1:
                    waits = list(si.on_wait)
                    for w in waits[:-1]:
                        nop = mybir.InstNoOp(
                            name=nc.get_next_instruction_name(),
                            engine=inst.engine,
                            bass_nofuse=True,
                            ins=[],
                            outs=[],
                        )
                        nop.sync_info = mybir.SyncInfo(on_wait=[w], on_update=[])
                        out.append(nop)
                    inst.sync_info = mybir.SyncInfo(
                        on_wait=[waits[-1]], on_update=list(si.on_update)
                    )
                    changed = True
                out.append(inst)
            if changed:
                bb.instructions = out
BF16 = mybir.dt.bfloat16
F32 = mybir.dt.float32
I32 = mybir.dt.int32
AF = mybir.ActivationFunctionType
ALU = mybir.AluOpType
AX = mybir.AxisListType

# problem dims
B, S, D, F, E, K = 8, 2048, 256, 1024, 8, 2
NCORES = 8
T = B * S // NCORES          # tokens per core = 2048
NT = T // 128                # token tiles per core = 16
KD = D // 128                # k-tiles over D = 2
NF = F // 128                # f-tiles over F = 8
CAP = 768                    # per-expert token capacity (mean is 512)
NS = CAP // 128              # slot tiles per expert = 6

MODE = "dense"              # "dense" or "sparse"
SKIP_SCATTER = False         # debug: drop x-row scatters
SKIP_GATHER = False          # debug: drop combine gathers
SKIP_EXPERTS = False         # debug: drop expert compute loop

bf16 = ml_dtypes.bfloat16


def _to_bf(a):
    return np.ascontiguousarray(a.astype(bf16))


def _to_f32(a):
    return np.ascontiguousarray(a.astype(np.float32))


def make_core_inputs(x_shard, w):
    """x_shard: [T, D] fp32. w: dict of full weight arrays. Returns in_map."""
    xT = np.ascontiguousarray(x_shard.T)                     # [D, T]
    xt32 = xT.reshape(KD, 128, T).transpose(1, 0, 2).reshape(128, KD * T)
    m = {
        "xt32": _to_f32(xt32),
        "xtb": _to_bf(xt32),
        "wr_t": _to_f32(w["Wr"].reshape(KD, 128, E).transpose(1, 0, 2).reshape(128, KD * E)),
        "w1s_t": _to_bf(w["W1s"].reshape(KD, 128, F).transpose(1, 0, 2).reshape(128, KD * F)),
        "w2s_t": _to_bf(w["W2s"].reshape(NF, 128, D).transpose(1, 0, 2).reshape(128, NF * D)),
        "w1p_t": _to_bf(w["W1p"].reshape(E, KD, 128, F).transpose(2, 0, 1, 3).reshape(128, E * KD * F)),
        "w2p_t": _to_bf(w["W2p"].reshape(E, NF, 128, D).transpose(2, 0, 1, 3).reshape(128, E * NF * D)),
        "b1s_t": _to_f32(w["b1s"].reshape(NF, 128).T),
        "b1p_t": _to_f32(w["b1p"].reshape(E, NF, 128).transpose(2, 0, 1).reshape(128, E * NF)),
        "b2s_r": _to_bf(w["b2s"].reshape(1, D)),
        "b2p_r": _to_bf(w["b2p"].reshape(1, E * D)),
        "ones_b": np.ones((1, 128), dtype=bf16),
    }
    if MODE == "sparse":
        m["xb"] = _to_bf(x_shard)                            # [T, D] token-major
        m["ones_f"] = np.ones((1, 128), dtype=np.float32)
        m["onec_f"] = np.ones((128, 1), dtype=np.float32)
        ks, ms = np.meshgrid(np.arange(128), np.arange(128), indexing="ij")
        m["tri_s"] = _to_f32(ks < ms)                        # lhsT[k,m]=1 iff k<m
        m["ecp1"] = _to_f32(np.broadcast_to(np.arange(E) * CAP + 1.0, (128, E)))
    return m


def build_nc(body_reps=1):
    nc = bass.Bass()
    # inputs
    xt32 = nc.declare_dram_parameter("xt32", [128, KD * T], F32, isOutput=False)
    xtb = nc.declare_dram_parameter("xtb", [128, KD * T], BF16, isOutput=False)
    wr_t = nc.declare_dram_parameter("wr_t", [128, KD * E], F32, isOutput=False)
    w1s_t = nc.declare_dram_parameter("w1s_t", [128, KD * F], BF16, isOutput=False)
    w2s_t = nc.declare_dram_parameter("w2s_t", [128, NF * D], BF16, isOutput=False)
    w1p_t = nc.declare_dram_parameter("w1p_t", [128, E * KD * F], BF16, isOutput=False)
    w2p_t = nc.declare_dram_parameter("w2p_t", [128, E * NF * D], BF16, isOutput=False)
    b1s_t = nc.declare_dram_parameter("b1s_t", [128, NF], F32, isOutput=False)
    b1p_t = nc.declare_dram_parameter("b1p_t", [128, E * NF], F32, isOutput=False)
    b2s_r = nc.declare_dram_parameter("b2s_r", [1, D], BF16, isOutput=False)
    b2p_r = nc.declare_dram_parameter("b2p_r", [1, E * D], BF16, isOutput=False)
    ones_b = nc.declare_dram_parameter("ones_b", [1, 128], BF16, isOutput=False)
    if MODE == "sparse":
        xb = nc.declare_dram_parameter("xb", [T, D], BF16, isOutput=False)
        ones_f = nc.declare_dram_parameter("ones_f", [1, 128], F32, isOutput=False)
        onec_f = nc.declare_dram_parameter("onec_f", [128, 1], F32, isOutput=False)
        tri_s = nc.declare_dram_parameter("tri_s", [128, 128], F32, isOutput=False)
        ecp1 = nc.declare_dram_parameter("ecp1", [128, E], F32, isOutput=False)
    y = nc.declare_dram_parameter("y", [T, D], F32, isOutput=True)

    with tile.TileContext(nc) as tc:
        with (
            tc.tile_pool(name="const", bufs=1) as cpool,
            tc.tile_pool(name="hts", bufs=1) as hts_pool,
            tc.tile_pool(name="hte", bufs=(2 if MODE == "sparse" else 1)) as hte_pool,
            tc.tile_pool(name="acc", bufs=1) as acc_pool,
            tc.tile_pool(name="rsm", bufs=2) as rsm,
            tc.tile_pool(name="psr", bufs=3, space="PSUM") as psr,
            tc.tile_pool(name="ps1", bufs=1, space="PSUM") as ps1,
            tc.tile_pool(name="psh", bufs=2, space="PSUM") as psh,
            tc.tile_pool(name="psc", bufs=2, space="PSUM") as psc,
            tc.tile_pool(name="outp", bufs=6) as outp,
        ):
            # ---- load constants / weights / activations into SBUF
            def load(pool, src, shape, dtype):
                t = pool.tile(shape, dtype, tag=f"c_{src.name}")
                nc.sync.dma_start(t[:], src[:])
                return t

            xt32_s = load(cpool, xt32, [128, KD * T], F32)
            xtb_s = load(cpool, xtb, [128, KD * T], BF16)
            wr_s = load(cpool, wr_t, [128, KD * E], F32)
            w1s_s = load(cpool, w1s_t, [128, KD * F], BF16)
            w2s_s = load(cpool, w2s_t, [128, NF * D], BF16)
            w1p_s = load(cpool, w1p_t, [128, E * KD * F], BF16)
            w2p_s = load(cpool, w2p_t, [128, E * NF * D], BF16)
            b1s_s = load(cpool, b1s_t, [128, NF], F32)
            b1p_s = load(cpool, b1p_t, [128, E * NF], F32)
            b2s_s = load(cpool, b2s_r, [1, D], BF16)
            b2p_s = load(cpool, b2p_r, [1, E * D], BF16)
            ones_s = load(cpool, ones_b, [1, 128], BF16)
            if MODE == "sparse":
                onesf_s = load(cpool, ones_f, [1, 128], F32)
                onecf_s = load(cpool, onec_f, [128, 1], F32)
                tri_s_s = load(cpool, tri_s, [128, 128], F32)
                ecp1_s = load(cpool, ecp1, [128, E], F32)
                # DRAM scratch
                with tc.tile_pool(name="dscr", bufs=1, space="DRAM") as dpool:
                    xg_d = dpool.tile([E * CAP, D], BF16)
                    eo_d = dpool.tile([E * CAP, D], BF16)
                    env = locals()
                    for _rep in range(body_reps):
                        run_sparse(nc, tc, env)
            else:
                env = locals()
                for _rep in range(body_reps):
                    run_dense(nc, tc, env)
    _split_multi_waits(nc)
    return nc


def router_tile(nc, t, env, rsm, want_dense_wn):
    """Router for token tile t. Returns dict with per-tile router tiles."""
    psr = env["psr"]
    xt32_s, wr_s = env["xt32_s"], env["wr_s"]
    pr = psr.tile([128, E], F32, tag="pr")
    for kd in range(KD):
        nc.tensor.matmul(
            pr[:],
            xt32_s[:, kd * T + t * 128 : kd * T + (t + 1) * 128],
            wr_s[:, kd * E : (kd + 1) * E],
            start=(kd == 0),
            stop=(kd == KD - 1),
        )
    negmax = rsm.tile([128, 1], F32, tag="negmax")
    nc.vector.tensor_reduce(negmax[:], pr[:], axis=AX.X, op=ALU.max)
    nc.vector.tensor_scalar_mul(negmax[:], negmax[:], -1.0)
    u = rsm.tile([128, E], F32, tag="u")
    nc.scalar.activation(u[:], pr[:], AF.Exp, bias=negmax[:, 0:1])
    s8 = rsm.tile([128, 8], F32, tag="s8")
    nc.vector.max(out=s8[:], in_=u[:])
    nc.vector.memset(s8[:, K:8], 0.0)
    zap = rsm.tile([128, E], F32, tag="zap")
    nc.vector.match_replace(out=zap[:], in_to_replace=s8[:], in_values=u[:], imm_value=0.0)
    wm = rsm.tile([128, E], F32, tag="wm")
    nc.vector.tensor_sub(wm[:], u[:], zap[:])
    out = {"wm": wm}
    if want_dense_wn:
        ssum = rsm.tile([128, 1], F32, tag="ssum")
        nc.vector.tensor_reduce(ssum[:], wm[:], axis=AX.X, op=ALU.add)
        rs = rsm.tile([128, 1], F32, tag="rs")
        nc.vector.reciprocal(rs[:], ssum[:])
        wn = rsm.tile([128, E], F32, tag="wn")
        nc.vector.tensor_scalar(wn[:], wm[:], rs[:, 0:1], None, op0=ALU.mult)
        out["wn"] = wn
    return out


def shared_l1(nc, env):
    """Shared expert first layer -> hTs [128f, NF x T] bf16 (persistent)."""
    psh, hts_pool = env["psh"], env["hts_pool"]
    xtb_s, w1s_s, b1s_s = env["xtb_s"], env["w1s_s"], env["b1s_s"]
    hts = hts_pool.tile([128, NF * T], BF16)
    for c in range(T // 512):
        for j in range(NF):
            ph = psh.tile([128, 512], F32, tag="ph")
            for kd in range(KD):
                nc.tensor.matmul(
                    ph[:],
                    w1s_s[:, kd * F + j * 128 : kd * F + (j + 1) * 128],
                    xtb_s[:, kd * T + c * 512 : kd * T + (c + 1) * 512],
                    start=(kd == 0),
                    stop=(kd == KD - 1),
                )
            nc.scalar.activation(
                hts[:, j * T + c * 512 : j * T + c * 512 + 512],
                ph[:],
                AF.Gelu,
                bias=b1s_s[:, j : j + 1],
            )
    return hts


def shared_l2_tile(nc, env, hts, t):
    """Shared expert second layer for token tile t -> psum [128tok, D]."""
    psc = env["psc"]
    w2s_s, b2s_s, ones_s = env["w2s_s"], env["b2s_s"], env["ones_s"]
    pc = psc.tile([128, D], F32, tag="pc")
    for j in range(NF):
        nc.tensor.matmul(
            pc[:],
            hts[:, j * T + t * 128 : j * T + (t + 1) * 128],
            w2s_s[:, j * D : (j + 1) * D],
            start=(j == 0),
            stop=False,
        )
    nc.tensor.matmul(pc[:], ones_s[:, :], b2s_s[:, :], start=False, stop=True)
    return pc


def run_dense(nc, tc, env):
    psh, psc = env["psh"], env["psc"]
    rsm, outp = env["rsm"], env["outp"]
    acc_pool, hte_pool = env["acc_pool"], env["hte_pool"]
    xtb_s = env["xtb_s"]
    w1p_s, w2p_s, b1p_s, b2p_s = env["w1p_s"], env["w2p_s"], env["b1p_s"], env["b2p_s"]
    ones_s = env["ones_s"]
    y = env["y"]

    # router for all tiles; keep dense renormalized weights
    wn_all = acc_pool.tile([128, NT * E], F32, tag="wn_all")
    for t in range(NT):
        r = router_tile(nc, t, env, rsm, want_dense_wn=True)
        nc.vector.tensor_copy(wn_all[:, t * E : (t + 1) * E], r["wn"][:])

    hts = shared_l1(nc, env)
    acc = acc_pool.tile([128, NT * D], F32, tag="acc")
    for t in range(NT):
        pc = shared_l2_tile(nc, env, hts, t)
        nc.vector.tensor_copy(acc[:, t * D : (t + 1) * D], pc[:])

    for e in range(E):
        hte = hte_pool.tile([128, NF * T], BF16, tag="hte")
        for c in range(T // 512):
            for j in range(NF):
                ph = psh.tile([128, 512], F32, tag="ph")
                for kd in range(KD):
                    nc.tensor.matmul(
                        ph[:],
                        w1p_s[:, (e * KD + kd) * F + j * 128 : (e * KD + kd) * F + (j + 1) * 128],
                        xtb_s[:, kd * T + c * 512 : kd * T + (c + 1) * 512],
                        start=(kd == 0),
                        stop=(kd == KD - 1),
                    )
                nc.scalar.activation(
                    hte[:, j * T + c * 512 : j * T + c * 512 + 512],
                    ph[:],
                    AF.Gelu,
                    bias=b1p_s[:, e * NF + j : e * NF + j + 1],
                )
        for t in range(NT):
            pe = psc.tile([128, D], F32, tag="pc")
            for j in range(NF):
                nc.tensor.matmul(
                    pe[:],
                    hte[:, j * T + t * 128 : j * T + (t + 1) * 128],
                    w2p_s[:, (e * NF + j) * D : (e * NF + j + 1) * D],
                    start=(j == 0),
                    stop=False,
                )
            nc.tensor.matmul(pe[:], ones_s[:, :], b2p_s[0:1, e * D : (e + 1) * D], start=False, stop=True)
            # acc += wn[:, e] * pe
            nc.vector.scalar_tensor_tensor(
                out=acc[:, t * D : (t + 1) * D],
                in0=pe[:],
                scalar=wn_all[:, t * E + e : t * E + e + 1],
                in1=acc[:, t * D : (t + 1) * D],
                op0=ALU.mult,
                op1=ALU.add,
            )
    for t in range(NT):
        nc.sync.dma_start(y[t * 128 : (t + 1) * 128, :], acc[:, t * D : (t + 1) * D])


def run_sparse(nc, tc, env):
    psh, psc, psr = env["psh"], env["psc"], env["psr"]
    rsm, outp = env["rsm"], env["outp"]
    acc_pool, hte_pool = env["acc_pool"], env["hte_pool"]
    xtb_s = env["xtb_s"]
    w1p_s, w2p_s, b1p_s, b2p_s = env["w1p_s"], env["w2p_s"], env["b1p_s"], env["b2p_s"]
    ones_s, onesf_s, onecf_s = env["ones_s"], env["onesf_s"], env["onecf_s"]
    tri_s_s, ecp1_s = env["tri_s_s"], env["ecp1_s"]
    xb, xg_d, eo_d = env["xb"], env["xg_d"], env["eo_d"]
    y = env["y"]

    # persistent per-tile router outputs
    wn01 = acc_pool.tile([128, NT * 2], F32, tag="wn01")      # renorm weights slot0/1
    ai_tiles = []
    for _t in range(NT):
        ai_t = acc_pool.tile([128, 2], I32, tag=f"ai{_t}")
        ai_tiles.append(ai_t)
    m_all = acc_pool.tile([128, NT * E], F32, tag="m_all")    # top2 selection masks
    wm_all = acc_pool.tile([128, NT * E], F32, tag="wm_all")  # masked unnorm weights
    pin_all = acc_pool.tile([128, NT * E], F32, tag="pin_all")  # intra pos + e*CAP + 1
    crow = acc_pool.tile([1, NT * E], F32, tag="crow")        # per-tile expert counts
    bases = acc_pool.tile([1, NT * E], F32, tag="bases")      # exclusive tile bases

    # pass A: router + intra-tile positions
    for t in range(NT):
        r = router_tile(nc, t, env, rsm, want_dense_wn=False)
        wm = r["wm"]
        me = m_all[:, t * E : (t + 1) * E]
        nc.vector.tensor_scalar(me, wm[:], 0.0, None, op0=ALU.is_gt)
        nc.vector.tensor_copy(wm_all[:, t * E : (t + 1) * E], wm[:])
        pp = psr.tile([128, E], F32, tag="pr")
        nc.tensor.matmul(pp[:], tri_s_s[:, :], me, start=True, stop=True)
        # pin = intra_pos + e*CAP + 1
        nc.vector.tensor_add(pin_all[:, t * E : (t + 1) * E], pp[:], ecp1_s[:])

    # per-tile expert counts in one matmul, then DVE-only prefix chain
    pcnt = env["ps1"].tile([1, NT * E], F32, tag="pcnt")
    nc.tensor.matmul(pcnt[:], onecf_s[:, :], m_all[:, :], start=True, stop=True)
    nc.vector.tensor_copy(crow[:], pcnt[:])
    nc.vector.memset(bases[:, 0:E], 0.0)
    for t in range(1, NT):
        nc.vector.tensor_add(
            bases[:, t * E : (t + 1) * E],
            bases[:, (t - 1) * E : t * E],
            crow[:, (t - 1) * E : t * E],
        )

    # pass B: global addresses, top-2 slot extraction, x-row scatter
    bc_reg = nc.gpsimd.to_reg(E * CAP - 1)
    for t in range(NT):
        bb = psr.tile([128, E], F32, tag="pr")
        nc.tensor.matmul(
            bb[:], onesf_s[:, :], bases[0:1, t * E : (t + 1) * E], start=True, stop=True
        )
        addr1 = rsm.tile([128, E], F32, tag="addr1")
        nc.vector.tensor_add(addr1[:], pin_all[:, t * E : (t + 1) * E], bb[:])
        key1 = rsm.tile([128, E], F32, tag="key1")
        nc.vector.tensor_mul(key1[:], m_all[:, t * E : (t + 1) * E], addr1[:])
        key2 = rsm.tile([128, E], F32, tag="key2")
        nc.vector.scalar_tensor_tensor(
            out=key2[:],
            in0=wm_all[:, t * E : (t + 1) * E],
            scalar=0.5,
            in1=key1[:],
            op0=ALU.mult,
            op1=ALU.add,
        )
        s1 = rsm.tile([128, 8], F32, tag="s1")
        s2 = rsm.tile([128, 8], F32, tag="s2")
        nc.vector.max(out=s1[:], in_=key1[:])
        nc.vector.max(out=s2[:], in_=key2[:])
        dk = rsm.tile([128, 2], F32, tag="dk")
        nc.vector.tensor_sub(dk[:], s2[:, 0:2], s1[:, 0:2])
        ssum = rsm.tile([128, 1], F32, tag="ssum")
        nc.vector.tensor_add(ssum[:], dk[:, 0:1], dk[:, 1:2])
        rs = rsm.tile([128, 1], F32, tag="rs")
        nc.vector.reciprocal(rs[:], ssum[:])
        nc.vector.tensor_scalar(
            wn01[:, t * 2 : t * 2 + 2], dk[:], rs[:, 0:1], None, op0=ALU.mult
        )
        a01 = rsm.tile([128, 2], F32, tag="a01")
        nc.vector.tensor_scalar(a01[:], s1[:, 0:2], 1.0, None, op0=ALU.subtract)
        nc.vector.tensor_copy(ai_tiles[t][:], a01[:])
        # scatter this tile's x rows into both expert slots
        if not SKIP_SCATTER:
            xrow = outp.tile([128, D], BF16, tag="xrow")
            nc.sync.dma_start(xrow[:], xb[t * 128 : (t + 1) * 128, :])
            for k in range(2):
                nc.gpsimd.indirect_dma_start(
                    out=xg_d[:],
                    out_offset=IndirectOffsetOnAxis(ap=ai_tiles[t][:, k : k + 1], axis=0),
                    in_=xrow[:],
                    in_offset=None,
                    bounds_check=bc_reg,
                    oob_is_err=True,
                )

    # shared expert L1 (overlaps with router/scatter on other engines)
    hts = shared_l1(nc, env)

    # private experts on gathered tokens
    for e in range([], range(E))[0 if SKIP_EXPERTS else 1] if False else (range(0) if SKIP_EXPERTS else range(E)):
        xgt = hte_pool.tile([128, KD * CAP], BF16, tag="xgt")
        for kd in range(KD):
            nc.sync.dma_start(
                xgt[:, kd * CAP : (kd + 1) * CAP],
                xg_d[e * CAP : (e + 1) * CAP, kd * 128 : (kd + 1) * 128],
                transpose=True,
            )
        hte = hte_pool.tile([128, NF * CAP], BF16, tag="hte")
        for c0 in range(0, CAP, 512):
            cn = min(512, CAP - c0)
            for j in range(NF):
                ph = psh.tile([128, 512], F32, tag="ph")
                for kd in range(KD):
                    nc.tensor.matmul(
                        ph[:, 0:cn],
                        w1p_s[:, (e * KD + kd) * F + j * 128 : (e * KD + kd) * F + (j + 1) * 128],
                        xgt[:, kd * CAP + c0 : kd * CAP + c0 + cn],
                        start=(kd == 0),
                        stop=(kd == KD - 1),
                    )
                nc.scalar.activation(
                    hte[:, j * CAP + c0 : j * CAP + c0 + cn],
                    ph[:, 0:cn],
                    AF.Gelu,
                    bias=b1p_s[:, e * NF + j : e * NF + j + 1],
                )
        for s in range(NS):
            pe = psc.tile([128, D], F32, tag="pc")
            for j in range(NF):
                nc.tensor.matmul(
                    pe[:],
                    hte[:, j * CAP + s * 128 : j * CAP + (s + 1) * 128],
                    w2p_s[:, (e * NF + j) * D : (e * NF + j + 1) * D],
                    start=(j == 0),
                    stop=False,
                )
            nc.tensor.matmul(pe[:], ones_s[:, :], b2p_s[0:1, e * D : (e + 1) * D], start=False, stop=True)
            eo = outp.tile([128, D], BF16, tag="eo")
            nc.vector.tensor_copy(eo[:], pe[:])
            nc.sync.dma_start(eo_d[e * CAP + s * 128 : e * CAP + (s + 1) * 128, :], eo[:])

    # combine: shared L2 + two gathered expert rows per token
    for t in range(NT):
        pc = shared_l2_tile(nc, env, hts, t)
        if SKIP_GATHER:
            ot = outp.tile([128, D], F32, tag="ot")
            nc.vector.tensor_copy(ot[:], pc[:])
            nc.sync.dma_start(y[t * 128 : (t + 1) * 128, :], ot[:])
        else:
            g0 = outp.tile([128, D], BF16, tag="g0")
            g1 = outp.tile([128, D], BF16, tag="g1")
            nc.gpsimd.indirect_dma_start(
                out=g0[:],
                out_offset=None,
                in_=eo_d[:],
                in_offset=IndirectOffsetOnAxis(ap=ai_tiles[t][:, 0:1], axis=0),
            )
            nc.gpsimd.indirect_dma_start(
                out=g1[:],
                out_offset=None,
                in_=eo_d[:],
                in_offset=IndirectOffsetOnAxis(ap=ai_tiles[t][:, 1:2], axis=0),
            )
            c1 = outp.tile([128, D], F32, tag="c1")
            nc.vector.scalar_tensor_tensor(
                out=c1[:], in0=g0[:], scalar=wn01[:, t * 2 : t * 2 + 1],
                in1=pc[:], op0=ALU.mult, op1=ALU.add,
            )
            ot = outp.tile([128, D], F32, tag="ot")
            nc.vector.scalar_tensor_tensor(
                out=ot[:], in0=g1[:], scalar=wn01[:, t * 2 + 1 : t * 2 + 2],
                in1=c1[:], op0=ALU.mult, op1=ALU.add,
            )
            nc.sync.dma_start(y[t * 128 : (t + 1) * 128, :], ot[:])


_NC_CACHE = {}


def _get_nc(body_reps=1):
    if body_reps not in _NC_CACHE:
        _NC_CACHE[body_reps] = build_nc(body_reps)
    return _NC_CACHE[body_reps]


def _make_in_maps(inputs):
    x = np.asarray(inputs["x"], dtype=np.float32).reshape(B * S, D)
    w = {k: np.asarray(v, dtype=np.float32) for k, v in inputs.items() if k != "x"}
    return [make_core_inputs(x[i * T : (i + 1) * T], w) for i in range(NCORES)]


def run(inputs, trace=False):
    nc = _get_nc()
    in_maps = _make_in_maps(inputs)
    res = run_bass_kernel_spmd(nc, in_maps, list(range(NCORES)), trace=trace)
    out = np.concatenate([res.results[i]["y"] for i in range(NCORES)], axis=0)
    return out.reshape(B, S, D), res



def bench(inputs, iters=8, reps=3, nc=None, in_maps=None, body_reps=1):
    """Marginal per-execution device time: `iters` chained executions
    (outputs donated forward), minus per-call dispatch measured separately."""
    import time as _time

    import jax
    import numpy as _np
    from jax.experimental.shard_map import shard_map
    from jax.sharding import Mesh, PartitionSpec

    from concourse import bass2jax

    if nc is None:
        nc = _get_nc(body_reps)
    if in_maps is None:
        in_maps = _make_in_maps(inputs)
    n_cores = NCORES

    in_names, out_names, out_avals, zero_outs = [], [], [], []
    for alloc in nc.m.functions[0].allocations:
        if not isinstance(alloc, mybir.MemoryLocationSet):
            continue
        name = alloc.memorylocations[0].name
        if alloc.kind == "ExternalInput":
            if nc.partition_id_tensor is None or name != nc.partition_id_tensor.name:
                in_names.append(name)
        elif alloc.kind == "ExternalOutput":
            shape = tuple(alloc.tensor_shape)
            dtype = mybir.dt.np(alloc.dtype)
            out_names.append(name)
            out_avals.append(jax.core.ShapedArray(shape, dtype))
            zero_outs.append(_np.zeros(shape, dtype))
    n_params = len(in_names)
    all_names = in_names + out_names
    if nc.partition_id_tensor is not None:
        all_names = all_names + [nc.partition_id_tensor.name]

    def _body(*args):
        ops = list(args)
        ins, outs = ops[:n_params], ops[n_params:]
        pid = (
            [bass2jax.partition_id_tensor()]
            if nc.partition_id_tensor is not None
            else []
        )
        outs = list(
            bass2jax._bass_exec_p.bind(
                *ins,
                *outs,
                *pid,
                out_avals=tuple(out_avals),
                in_names=tuple(all_names),
                out_names=tuple(out_names),
                lowering_input_output_aliases=(),
                sim_require_finite=True,
                sim_require_nnan=True,
                nc=nc,
            )
        )
        return tuple(outs)

    devices = jax.devices()[:n_cores]
    mesh = Mesh(_np.asarray(devices), ("core",))
    nin = n_params + len(zero_outs)
    fn = jax.jit(
        shard_map(
            _body,
            mesh=mesh,
            in_specs=(PartitionSpec("core"),) * nin,
            out_specs=(PartitionSpec("core"),) * len(out_names),
            check_rep=False,
        ),
        donate_argnums=tuple(range(n_params, nin)),
        keep_unused=True,
    )
    concat_in = [
        _np.concatenate([_np.asarray(in_maps[c][k]) for c in range(n_cores)], axis=0)
        for k in in_names
    ]
    shd = jax.sharding.NamedSharding(mesh, PartitionSpec("core"))
    dev_in = [jax.device_put(a, shd) for a in concat_in]
    outs = [
        _np.zeros((n_cores * z.shape[0], *z.shape[1:]), z.dtype) for z in zero_outs
    ]
    outs = list(fn(*dev_in, *outs))  # warmup (compile + upload)
    jax.block_until_ready(outs)
    result = [_np.asarray(o) for o in outs]
    times = []
    for _ in range(reps):
        t0 = _time.perf_counter()
        for _i in range(iters):
            outs = list(fn(*dev_in, *outs))
        jax.block_until_ready(outs)
        times.append(_time.perf_counter() - t0)
    return min(times), result


def make_tiny_nc():
    nc = bass.Bass()
    a = nc.declare_dram_parameter("a", [128, 8], F32, isOutput=False)
    z = nc.declare_dram_parameter("z", [128, 8], F32, isOutput=True)
    with tile.TileContext(nc) as tc:
        with tc.tile_pool(name="t", bufs=1) as tp:
            ta = tp.tile([128, 8], F32, tag="ta")
            nc.sync.dma_start(ta[:], a[:])
            nc.sync.dma_start(z[:], ta[:])
    _split_multi_waits(nc)
    return nc


def bench_floor(iters=8, reps=3):
    import numpy as _np

    nc = make_tiny_nc()
    maps = [{"a": _np.zeros((128, 8), _np.float32)} for _ in range(NCORES)]
    t, _ = bench(None, iters=iters, reps=reps, nc=nc, in_maps=maps)
    return t


# Capture the Tile scheduling sim's predicted end time (cost-model ns).
LAST_SIM_NS = [0.0]
import concourse.bass_interp as _bi

_ORIG_SIMULATE = _bi.CoreSim.simulate


def _rec_simulate(self, *a, **kw):
    r = _ORIG_SIMULATE(self, *a, **kw)
    try:
        t = float(self._sim_state.time)
        LAST_SIM_NS[0] = max(LAST_SIM_NS[0], t)
    except Exception:
        pass
    return r


_bi.CoreSim.simulate = _rec_simulate


def predicted_us(body_reps=1, fresh=True):
    if fresh:
        _NC_CACHE.pop(body_reps, None)
    LAST_SIM_NS[0] = 0.0
    _get_nc(body_reps)
    return LAST_SIM_NS[0] / 1000.0


def kernel(**inputs):
    out, _ = run(inputs, trace=False)
    return out
